# revision 3
# baseline (speedup 1.0000x reference)
"""Trainium2 Bass kernel v2 for segment max/mean pooling + Linear + ReLU.

Host reorders rows so that groups of EQUAL SIZE are contiguous ("runs"),
making segment reduction uniform-stride:

  - per 1024-row window (one PSUM bank): PE transposes bf16 tiles into the
    bank; DVE computes per-group max with a single strided tensor_reduce
    [128, G, s]; group MEANS come from PE matmuls against small constant
    block-one-hot tables (value 1/s), accumulated in a second PSUM tile.
  - per 128 groups: two accumulating matmuls (mx@W1t + mean@W2t) -> one
    PSUM tile -> ACT relu -> staged -> DMA out.

No scans, no gathers, no offsets. All data-dependent structure is resolved
on the host into a schedule that is IDENTICAL across the 8 cores (per-size
group counts are padded to multiples of 8 with dummy groups), so a single
SPMD program works; only the DMA'd data differs per core.

Input rows are pre-swizzled on the host into the exact on-device tile
layout, so lane DMAs are fully contiguous.
"""

from contextlib import ExitStack

import numpy as np

import concourse.bass as bass
import concourse.bacc as bacc
import concourse.tile as tile
from concourse import mybir
from concourse.bass_utils import run_bass_kernel_spmd

F32 = mybir.dt.float32
BF16 = mybir.dt.bfloat16

N_CORES = 8
D = 128
OUT = 128
WIN = 1024          # rows per window = one PSUM bank of bf16
TPW = WIN // 128    # tiles per window
GRAN_T = 64         # tiles per DMA granule (2 MiB of bf16)
GRAN = GRAN_T * 128
PSM_COLS = 500      # max real group-columns per window (bank limit margin)


# ----------------------------------------------------------------------------
# Host-side planning
# ----------------------------------------------------------------------------

def make_schedule(count8):
    """count8: {size: per-core group count} (identical across cores).

    Returns dict with:
      slots:    list of (s,) group sizes in placement order
      segs:     list of (s, G, row0, col_base)  [row0 absolute, 128-aligned]
      windows:  list over windows of dict(segs=[seg indices], psmw=int)
      tiles:    list over all tiles of (seg_idx, t_rel) or None (dead)
      ROWS, NWIN, NGRAN, NG (scheduled groups incl dummies), NG_PAD, NFT
    """
    segs = []
    slots = []
    win_cols = {}   # window -> columns used
    pos = 0

    def win_of(p):
        return p // WIN

    cur = None  # open segment [s, G, row0]

    def close():
        nonlocal cur
        if cur is not None and cur[1] > 0:
            segs.append(tuple(cur))
        cur = None

    cur_win = -1
    for s in sorted(count8):
        assert s <= WIN, f"group of size {s} cannot fit a window"
        n = count8[s]
        # run start: align to tile boundary
        if pos % 128 != 0:
            pos += 128 - pos % 128
        close()
        for _ in range(n):
            w = win_of(pos)
            # group must not cross a window boundary
            if pos + s > (w + 1) * WIN:
                pos = (w + 1) * WIN
                w = win_of(pos)
            # psm column budget per window
            if win_cols.get(w, 0) >= PSM_COLS:
                pos = (w + 1) * WIN
                w = win_of(pos)
            if cur is None or cur_win != w:
                close()
                cur = [s, 0, pos]
                cur_win = w
            cur[1] += 1
            win_cols[w] = win_cols.get(w, 0) + 1
            slots.append(s)
            pos += s
    close()

    ROWS = ((pos + GRAN - 1) // GRAN) * GRAN
    NWIN = ROWS // WIN
    NGRAN = ROWS // GRAN

    # per-window segment lists + psm column bases
    windows = [dict(segs=[], psmw=0) for _ in range(NWIN)]
    NG = 0
    segs2 = []
    for (s, G, row0) in segs:
        w = row0 // WIN
        assert (row0 + G * s - 1) // WIN == w
        base = windows[w]["psmw"]
        windows[w]["psmw"] = base + G + 3   # +3 margin for dead-row cols
        windows[w]["segs"].append(len(segs2))
        segs2.append(dict(s=s, G=G, row0=row0, col_base=base, gc=NG))
        NG += G
    for wd in windows:
        assert wd["psmw"] <= 512

    NFT = (NG + 127) // 128
    NG_PAD = NFT * 128

    # tile map: absolute tile -> (seg_idx, t_rel) or None
    tiles = []
    for t_abs in range(ROWS // 128):
        r = t_abs * 128
        owner = None
        for si in windows[r // WIN]["segs"]:
            sg = segs2[si]
            if sg["row0"] <= r < sg["row0"] + sg["G"] * sg["s"]:
                owner = (si, (r - sg["row0"]) // 128)
                break
        tiles.append(owner)

    return dict(slots=slots, segs=segs2, windows=windows, tiles=tiles,
                ROWS=ROWS, NWIN=NWIN, NGRAN=NGRAN, NG=NG, NG_PAD=NG_PAD,
                NFT=NFT)


def make_plan(seg_ids, n_cores=N_CORES):
    seg_ids = np.asarray(seg_ids).astype(np.int64)
    n_groups = int(seg_ids[-1]) + 1
    cnt = np.bincount(seg_ids, minlength=n_groups)
    assert cnt.min() >= 1
    gstarts = np.zeros(n_groups + 1, dtype=np.int64)
    np.cumsum(cnt, out=gstarts[1:])

    # groups by size -> deal round-robin to cores, pad with dummies (size s)
    order = np.argsort(cnt, kind="stable")          # group ids sorted by size
    sizes_sorted = cnt[order]
    count8 = {}
    assign = {s: [] for s in np.unique(sizes_sorted)}   # s -> group id list
    for g, s in zip(order, sizes_sorted):
        assign[int(s)].append(int(g))
    for s, gl in assign.items():
        n8 = ((len(gl) + n_cores - 1) // n_cores) * n_cores
        count8[s] = n8 // n_cores
        gl.extend([-1] * (n8 - len(gl)))            # -1 = dummy group

    sched = make_schedule(count8)

    # per-core slot -> group id (or -1); slots are emitted in ascending size
    core_slot_gid = [[] for _ in range(n_cores)]
    for s in sorted(count8):
        gl = assign[s]
        per = count8[s]
        for c in range(n_cores):
            core_slot_gid[c].extend(gl[c::n_cores])
        for c in range(n_cores):
            assert len(core_slot_gid[c]) == len(core_slot_gid[0])
    n_slots = len(sched["slots"])
    for c in range(n_cores):
        assert len(core_slot_gid[c]) == n_slots

    return dict(sched=sched, count8=count8, core_slot_gid=core_slot_gid,
                gstarts=gstarts, cnt=cnt, n_groups=n_groups)


def build_oh_tables(count8):
    """(s, phi) -> (offset, W); plus the concatenated [128, TOT] fp32 table."""
    idx = {}
    cols = []
    off = 0
    r = np.arange(128)
    for s in sorted(count8):
        for t in range(512):         # enough distinct phases: cycle len <= s
            phi = (128 * t) % s
            if (s, phi) in idx:
                if t > 0 and phi == 0:
                    break
                continue
            W = (phi + 127) // s + 1
            tab = np.zeros((128, W), dtype=np.float32)
            tab[r, (r + phi) // s] = 1.0 / s
            idx[(s, phi)] = (off, W)
            cols.append(tab)
            off += W
    return np.concatenate(cols, axis=1), idx, off


def make_inputs(plan, lane, W):
    sched = plan["sched"]
    ROWS, NGRAN = sched["ROWS"], sched["NGRAN"]
    oh_tab, oh_idx, oh_tot = build_oh_tables(plan["count8"])
    plan["oh_idx"] = oh_idx
    plan["oh_tot"] = oh_tot

    import ml_dtypes
    BF = ml_dtypes.bfloat16
    WT = np.ascontiguousarray(np.asarray(W, dtype=np.float32).T).astype(BF)
    ident = np.eye(128, dtype=np.float32).astype(BF)
    oh_tab = oh_tab.astype(BF)
    gstarts = plan["gstarts"]
    cnt = plan["cnt"]
    segs = sched["segs"]

    # build per-core row permutation (index M = zero row)
    M = lane.shape[0]
    lane_z = np.concatenate([lane, np.zeros((1, D), dtype=np.float32)])
    perm = np.full((N_CORES, ROWS), M, dtype=np.int64)
    slot_gid = np.asarray(plan["core_slot_gid"])      # [8, n_slots]
    si = 0
    for sg in segs:
        s, G, row0 = sg["s"], sg["G"], sg["row0"]
        gids = slot_gid[:, si:si + G]                  # [8, G]
        si += G
        r0s = np.where(gids >= 0, gstarts[np.maximum(gids, 0)], M)
        src = r0s[:, :, None] + np.arange(s)[None, None, :]
        src = np.where(gids[:, :, None] >= 0, src, M)
        perm[:, row0:row0 + G * s] = src.reshape(N_CORES, G * s)
    assert si == slot_gid.shape[1]

    in_maps = []
    for c in range(N_CORES):
        lanes = lane_z[perm[c]].astype(BF)
        lanes_dev = np.ascontiguousarray(
            lanes.reshape(NGRAN, GRAN_T, 128, D).transpose(0, 2, 1, 3))
        in_maps.append(dict(lanes=lanes_dev, ohtab=oh_tab,
                            wt=WT, ident=ident))
    return in_maps


# ----------------------------------------------------------------------------
# Device program
# ----------------------------------------------------------------------------

def build_nc(plan):
    sched = plan["sched"]
    ROWS, NWIN, NGRAN = sched["ROWS"], sched["NWIN"], sched["NGRAN"]
    NG, NG_PAD, NFT = sched["NG"], sched["NG_PAD"], sched["NFT"]
    segs, windows, tiles = sched["segs"], sched["windows"], sched["tiles"]
    oh_idx, oh_tot = plan["oh_idx"], plan["oh_tot"]

    psmw_max = max((wd["psmw"] for wd in windows if wd["segs"]), default=128)
    PSMW_T = min(512, ((psmw_max + 127) // 128) * 128)

    nc = bacc.Bacc("TRN2", target_bir_lowering=False, debug=False,
                   num_devices=N_CORES)
    lanes = nc.dram_tensor("lanes", [NGRAN, 128, GRAN_T, D], BF16,
                           kind="ExternalInput")
    ohtab = nc.dram_tensor("ohtab", [128, oh_tot], BF16, kind="ExternalInput")
    wt = nc.dram_tensor("wt", [2 * D, OUT], BF16, kind="ExternalInput")
    ident = nc.dram_tensor("ident", [128, 128], BF16, kind="ExternalInput")
    out_c = nc.dram_tensor("out_c", [NG_PAD, OUT], F32, kind="ExternalOutput")
    out_r = out_c[:, :].rearrange("(j p) o -> p j o", p=128)

    with tile.TileContext(nc) as tc, ExitStack() as ctx:
        consts = ctx.enter_context(tc.tile_pool(name="consts", bufs=1))
        flats = ctx.enter_context(tc.tile_pool(name="flats", bufs=1))
        xbpool = ctx.enter_context(tc.tile_pool(name="xbpool", bufs=6))
        stpool = ctx.enter_context(tc.tile_pool(name="stpool", bufs=3))
        psumT = ctx.enter_context(tc.tile_pool(name="psumT", bufs=4, space="PSUM"))
        psumM = ctx.enter_context(tc.tile_pool(name="psumM", bufs=3, space="PSUM"))
        psumF = ctx.enter_context(tc.tile_pool(name="psumF", bufs=1, space="PSUM"))

        ident_sb = consts.tile([128, 128], BF16)
        nc.sync.dma_start(out=ident_sb[:, :], in_=ident[:, :])
        oh_sb = consts.tile([128, oh_tot], BF16)
        nc.sync.dma_start(out=oh_sb[:, :], in_=ohtab[:, :])
        w1t_sb = consts.tile([128, OUT], BF16)
        nc.sync.dma_start(out=w1t_sb[:, :], in_=wt[0:128, :])
        w2t_sb = consts.tile([128, OUT], BF16)
        nc.sync.dma_start(out=w2t_sb[:, :], in_=wt[128:256, :])

        mx_flat = flats.tile([128, NG_PAD], BF16)
        mn_flat = flats.tile([128, NG_PAD], BF16)
        if NG < NG_PAD:  # dummy pad columns never written by reduce/copy
            nc.vector.memset(mx_flat[:, NG:], 0.0)
            nc.vector.memset(mn_flat[:, NG:], 0.0)

        ft_state = [0]

        def flush_fts(upto):
            while (ft_state[0] + 1) * 128 <= upto:
                j = ft_state[0]
                p_full = psumF.tile([128, 512], F32, tag="p")
                p = p_full[:, 0:OUT]
                nc.tensor.matmul(p[:, :], mx_flat[:, j * 128:(j + 1) * 128],
                                 w1t_sb[:, :], start=True, stop=False)
                nc.tensor.matmul(p[:, :], mn_flat[:, j * 128:(j + 1) * 128],
                                 w2t_sb[:, :], start=False, stop=True)
                stage = stpool.tile([128, OUT], F32, tag="stage")
                nc.scalar.activation(stage[:, :], p[:, :],
                                     mybir.ActivationFunctionType.Relu)
                nc.sync.dma_start(out=out_r[:, j, :], in_=stage[:, :])
                ft_state[0] += 1

        gc_done = 0
        for g in range(NGRAN):
            xb = xbpool.tile([128, GRAN_T, D], BF16, tag="xb")
            if g == 0:  # slice the first load so window 0 lands fast
                for q in range(0, GRAN_T, TPW):
                    nc.sync.dma_start(out=xb[:, q:q + TPW, :],
                                      in_=lanes[g][:, q:q + TPW, :])
            else:
                nc.sync.dma_start(out=xb[:, :, :], in_=lanes[g])
            for wloc in range(GRAN // WIN):
                w = g * (GRAN // WIN) + wloc
                wd = windows[w]
                if not wd["segs"]:
                    continue
                bank = psumT.tile([128, TPW, 128], BF16, tag="bank")
                psm_full = psumM.tile([128, 512], F32, tag="psm")
                psm = psm_full[:, 0:PSMW_T]
                # matmul start/stop bookkeeping across the window's oh matmuls
                live = [t for t in range(TPW)
                        if tiles[w * TPW + t] is not None]
                first_t, last_t = live[0], live[-1]
                for t in range(TPW):
                    owner = tiles[w * TPW + t]
                    if owner is None:
                        continue
                    si, t_rel = owner
                    sg = segs[si]
                    s = sg["s"]
                    xt = xb[:, wloc * TPW + t, :]
                    nc.tensor.transpose(bank[:, t, :], xt, ident_sb[:, :])
                    phi = (128 * t_rel) % s
                    off, W = oh_idx[(s, phi)]
                    g0 = sg["col_base"] + (128 * t_rel) // s
                    # clamp to the segment's real columns (beyond = dead rows)
                    W_eff = min(W, sg["col_base"] + sg["G"] - g0)
                    assert W_eff >= 1
                    nc.tensor.matmul(psm[:, g0:g0 + W_eff], xt,
                                     oh_sb[:, off:off + W_eff],
                                     start=(t == first_t), stop=(t == last_t))
                bank2d = bank[:, :, :].rearrange("p t d -> p (t d)")
                for si in wd["segs"]:
                    sg = segs[si]
                    s, G = sg["s"], sg["G"]
                    r_off = sg["row0"] - w * WIN
                    nc.vector.tensor_reduce(
                        out=mx_flat[:, sg["gc"]:sg["gc"] + G],
                        in_=bank2d[:, r_off:r_off + G * s].rearrange(
                            "p (gg ss) -> p gg ss", ss=s),
                        axis=mybir.AxisListType.X,
                        op=mybir.AluOpType.max)
                    nc.scalar.activation(
                        mn_flat[:, sg["gc"]:sg["gc"] + G],
                        psm[:, sg["col_base"]:sg["col_base"] + G],
                        mybir.ActivationFunctionType.Copy)
                    gc_done = sg["gc"] + G
                flush_fts(gc_done)

        flush_fts(NG_PAD)

    nc.finalize()
    return nc


# ----------------------------------------------------------------------------
# Entry point
# ----------------------------------------------------------------------------

LAST_RESULT = None


def kernel(obs_encoding, lane_encoding, same_obs_mask, W, b, _debug=None):
    global LAST_RESULT
    seg = np.asarray(same_obs_mask)[:, 0]
    lane = np.asarray(lane_encoding, dtype=np.float32)
    assert np.abs(np.asarray(b)).max() == 0.0, "nonzero bias not implemented"

    plan = make_plan(seg)
    in_maps = make_inputs(plan, lane, np.asarray(W))
    nc = build_nc(plan)
    kw = dict(_debug or {})
    res = run_bass_kernel_spmd(nc, in_maps, list(range(N_CORES)), **kw)
    LAST_RESULT = res

    n_groups = plan["n_groups"]
    out = np.zeros((n_groups, OUT), dtype=np.float32)
    for c in range(N_CORES):
        oc = res.results[c]["out_c"]
        gids = plan["core_slot_gid"][c]
        gids_a = np.asarray(gids)
        valid = gids_a >= 0
        out[gids_a[valid]] = oc[:len(gids)][valid]
    return out
